# revision 19
# baseline (speedup 1.0000x reference)
"""AssocScan Trainium2 kernel: out[:, t] = gates[:, t] * out[:, t-1] + inputs[:, t].

Strategy: the recurrence is independent per (b, d) lane (B*D = 4096 lanes,
N = 4096 steps). The DVE `tensor_tensor_scan` instruction computes exactly
this recurrence along the free dimension, one lane per partition, at a
measured ~2.09 ns/column — that serial rate is the kernel's hard floor
(no other engine supports the scan opcode on NeuronCore v3).

Layout: lanes are split across the 8 cores (512 lanes each). On the host,
each core's 512 lanes are packed 4-per-partition, concatenated along the
free dim into one [128, 16384] stream. Because g[:, 0] of every lane
never affects the result (it multiplies the zero initial state), the host
zeroes it; the scan state then self-resets at each lane boundary, so the
whole stream can be scanned with a handful of long chained
tensor_tensor_scan instructions (long scans amortize the per-instruction
overhead; chaining passes the carry through the previous segment's last
output column).

Pipeline: column segments [2048, 2048, 4096, 4096, 3584, 512] — a shorter
head so the first scan starts ~12.5 us in, big mid-stream segments (the
load ring saturates ~330-370 B/ns only with few large transfers; the scan
consumes ~245 B/ns so loads stay ahead), and a short tail so the final
store is brief. All loads ride the sync-engine HWDGE ring in FIFO scan
order, except the first segment's inputs, which go on the scalar ring so
both queues ramp concurrently. Stores ride the scalar ring.

Measured (trace-on): ~53.5 us vs 64.7 us for the prior chunked kernel on
the same measurement path; the scan itself accounts for ~35.5 us and is
the hardware floor.
"""

import sys

import numpy as np

for _p in ("/opt/trn_rl_repo", "/opt/pypackages"):
    if _p not in sys.path:
        sys.path.append(_p)

import concourse.bacc as bacc
import concourse.mybir as mybir
from concourse.bass_utils import run_bass_kernel_spmd
from concourse.tile import TileContext

B, N, D = 4, 4096, 1024
N_CORES = 8
LANES = B * D                        # 4096 independent (b, d) lanes
LANES_PER_CORE = LANES // N_CORES    # 512
P = 128                              # SBUF partitions
LPP = LANES_PER_CORE // P            # 4 lanes per partition
NC = LPP * N                         # 16384 columns per partition

TRACE = False       # test harness sets True to capture a neuron-profile trace
USE_BF16 = True     # bf16 inputs: quantization ~2e-3 rel, halves load bytes
BF16_OUT = True     # bf16 output stores: halves store bytes
_result_info = {}   # exec_time_ns / trace path from the last run

# Column segment sizes (sum = NC). Small head segments let the scan start
# as soon as the first columns land; the short tail segment keeps the
# final store off the critical path.
import os as _os

_SEGS = [int(s) for s in _os.environ.get(
    "SEGS", "2048,2048,4096,4096,3584,512"
).split(",")]
assert sum(_SEGS) == NC
_STORE_ENG = _os.environ.get("STORE_ENG", "scalar")
# Segments >= this index load g and x packed in a single transfer
# (measured no faster than two 8KB-line transfers; disabled by default).
_PACK_FROM = int(_os.environ.get("PACK_FROM", "99"))


def _build() -> bacc.Bacc:
    in_dt = mybir.dt.bfloat16 if USE_BF16 else mybir.dt.float32
    out_dt = mybir.dt.bfloat16 if BF16_OUT else mybir.dt.float32
    nc = bacc.Bacc()
    # One contiguous DRAM tensor per segment: every DMA source/dest is a
    # single dense block, which keeps the queues at full descriptor
    # efficiency (column-slicing one big [P, NC] tensor dropped the load
    # rings to ~140 B/ns; dense blocks run ~290 B/ns). Segments >= _PACK_FROM
    # pack g and x into one [P, 2*seg] tensor (row = g||x): double the
    # per-partition line length and half the transfer count.
    gs, xs, gxs = [], [], []
    for k, seg in enumerate(_SEGS):
        if k >= _PACK_FROM:
            gxs.append(
                nc.dram_tensor(f"gx{k}", [P, 2 * seg], in_dt, kind="ExternalInput")
            )
            gs.append(None)
            xs.append(None)
        else:
            gxs.append(None)
            gs.append(
                nc.dram_tensor(f"g{k}", [P, seg], in_dt, kind="ExternalInput")
            )
            xs.append(
                nc.dram_tensor(f"x{k}", [P, seg], in_dt, kind="ExternalInput")
            )
    os_ = [
        nc.dram_tensor(f"o{k}", [P, seg], out_dt, kind="ExternalOutput")
        for k, seg in enumerate(_SEGS)
    ]
    M = mybir.AluOpType.mult
    A = mybir.AluOpType.add
    with TileContext(nc) as tc:
        with tc.tile_pool(name="pool", bufs=1) as pool:
            gts, xts, gxts, ots = [], [], [], []
            for k, seg in enumerate(_SEGS):
                if k >= _PACK_FROM:
                    gxt = pool.tile([P, 2 * seg], in_dt, name=f"gxt{k}")
                    gxts.append(gxt)
                    gts.append(gxt[:, 0:seg])
                    xts.append(gxt[:, seg : 2 * seg])
                else:
                    gxts.append(None)
                    gts.append(pool.tile([P, seg], in_dt, name=f"gt{k}"))
                    xts.append(pool.tile([P, seg], in_dt, name=f"xt{k}"))
                ots.append(pool.tile([P, seg], out_dt, name=f"ot{k}"))
            # Issue every load up front: g on the sync HWDGE ring, x on the
            # scalar HWDGE ring. All tiles coexist in SBUF (96 KiB/partition),
            # so nothing waits on a buffer release.
            x_head = int(_os.environ.get("X_HEAD", "2"))
            for k, seg in enumerate(_SEGS):
                if k >= _PACK_FROM:
                    nc.sync.dma_start(out=gxts[k][:, :], in_=gxs[k][:, :])
                elif k < x_head:
                    # Head: g and x on different rings so both queues ramp
                    # concurrently and the first segment lands sooner.
                    nc.sync.dma_start(out=gts[k][:, :], in_=gs[k][:, :])
                    nc.scalar.dma_start(out=xts[k][:, :], in_=xs[k][:, :])
                else:
                    nc.sync.dma_start(out=gts[k][:, :], in_=gs[k][:, :])
                    nc.sync.dma_start(out=xts[k][:, :], in_=xs[k][:, :])
            # Chained scans; carry crosses segment boundaries through the
            # previous segment's last output column (bf16 rounding there is
            # far inside the error budget). Lane resets happen wherever the
            # host zeroed the gate. Stores ride the gpsimd SWDGE ring so
            # they never contend with load dispatch.
            engs = {
                "gpsimd": nc.gpsimd,
                "scalar": nc.scalar,
                "sync": nc.sync,
            }
            store_plan = _os.environ.get("STORE_PLAN", "")
            if store_plan:
                store_engs = [engs[s] for s in store_plan.split(",")]
                assert len(store_engs) == len(_SEGS)
            else:
                store_engs = [engs[_STORE_ENG]] * len(_SEGS)
            for k, seg in enumerate(_SEGS):
                init = 0.0 if k == 0 else ots[k - 1][:, -1:]
                nc.vector.tensor_tensor_scan(
                    ots[k][:, :], gts[k][:, :], xts[k][:, :], init, M, A
                )
                store_engs[k].dma_start(out=os_[k][:, :], in_=ots[k][:, :])
    nc.compile()
    return nc


def kernel(gates: np.ndarray, inputs: np.ndarray) -> np.ndarray:
    gates = np.asarray(gates, dtype=np.float32)
    inputs = np.asarray(inputs, dtype=np.float32)

    # Host-side shard: (B, N, D) -> lane-major (B*D, N); row b*D + d is the
    # contiguous time series of lane (b, d). The first gate of every lane
    # multiplies the zero initial state, so it is dead — zero it to make
    # the scan state reset at lane boundaries after concatenation.
    gt = np.ascontiguousarray(gates.transpose(0, 2, 1)).reshape(LANES, N)
    xt = np.ascontiguousarray(inputs.transpose(0, 2, 1)).reshape(LANES, N)
    gt[:, 0] = 0.0
    if USE_BF16:
        import ml_dtypes

        gt = gt.astype(ml_dtypes.bfloat16)
        xt = xt.astype(ml_dtypes.bfloat16)

    # Per core: [512, N] -> [LPP, P, N] -> [P, LPP, N] -> [P, NC]: partition
    # p holds lanes {base + p, base + P + p, ...} concatenated in time.
    # Each column segment ships as its own contiguous array.
    bounds = np.cumsum([0] + _SEGS)
    in_maps = []
    for c in range(N_CORES):
        rows = slice(c * LANES_PER_CORE, (c + 1) * LANES_PER_CORE)
        gc = gt[rows].reshape(LPP, P, N).transpose(1, 0, 2).reshape(P, NC)
        xc = xt[rows].reshape(LPP, P, N).transpose(1, 0, 2).reshape(P, NC)
        m = {}
        for k in range(len(_SEGS)):
            sl = slice(bounds[k], bounds[k + 1])
            if k >= _PACK_FROM:
                m[f"gx{k}"] = np.ascontiguousarray(
                    np.concatenate([gc[:, sl], xc[:, sl]], axis=1)
                )
            else:
                m[f"g{k}"] = np.ascontiguousarray(gc[:, sl])
                m[f"x{k}"] = np.ascontiguousarray(xc[:, sl])
        in_maps.append(m)

    nc = _build()
    res = run_bass_kernel_spmd(
        nc, in_maps, core_ids=list(range(N_CORES)), trace=TRACE
    )
    _result_info["exec_time_ns"] = res.exec_time_ns
    _result_info["mean_exec_time_ns"] = res.mean_exec_time_ns
    _result_info["profile_json"] = res.profile_json
    _result_info["trace"] = (
        res.instructions_and_trace[1] if res.instructions_and_trace else None
    )

    # Undo the per-core packing: concat segments -> [P, NC] -> [P, LPP, N]
    # -> [LPP, P, N] -> [512, N], then stack cores back to (LANES, N).
    parts = []
    for c in range(N_CORES):
        oc = np.concatenate(
            [
                res.results[c][f"o{k}"].astype(np.float32, copy=False)
                for k in range(len(_SEGS))
            ],
            axis=1,
        )
        parts.append(
            oc.reshape(P, LPP, N).transpose(1, 0, 2).reshape(LANES_PER_CORE, N)
        )
    out_t = np.concatenate(parts, axis=0)  # (LANES, N)
    return np.ascontiguousarray(out_t.reshape(B, D, N).transpose(0, 2, 1))


# revision 21
# speedup vs baseline: 1.0271x; 1.0271x over previous
"""AssocScan Trainium2 kernel: out[:, t] = gates[:, t] * out[:, t-1] + inputs[:, t].

Strategy: the recurrence is independent per (b, d) lane (B*D = 4096 lanes,
N = 4096 steps). The DVE `tensor_tensor_scan` instruction computes exactly
this recurrence along the free dimension, one lane per partition, at a
measured ~2.09 ns/column — that serial rate is the kernel's hard floor
(no other engine supports the scan opcode on NeuronCore v3).

Layout: lanes are split across the 8 cores (512 lanes each). On the host,
each core's 512 lanes are packed 4-per-partition, concatenated along the
free dim into one [128, 16384] stream. Because g[:, 0] of every lane
never affects the result (it multiplies the zero initial state), the host
zeroes it; the scan state then self-resets at each lane boundary, so the
whole stream can be scanned with a handful of long chained
tensor_tensor_scan instructions (long scans amortize the per-instruction
overhead; chaining passes the carry through the previous segment's last
output column).

Pipeline: column segments [2048, 2048, 4096, 4096, 3584, 512] — a shorter
head so the first scan starts ~12.5 us in, big mid-stream segments (the
load ring saturates ~330-370 B/ns only with few large transfers; the scan
consumes ~245 B/ns so loads stay ahead), and a short tail so the final
store is brief. All loads ride the sync-engine HWDGE ring in FIFO scan
order, except the first segment's inputs, which go on the scalar ring so
both queues ramp concurrently. Stores ride the scalar ring.

Measured (trace-on): ~53.5 us vs 64.7 us for the prior chunked kernel on
the same measurement path; the scan itself accounts for ~35.5 us and is
the hardware floor.
"""

import sys

import numpy as np

for _p in ("/opt/trn_rl_repo", "/opt/pypackages"):
    if _p not in sys.path:
        sys.path.append(_p)

import concourse.bacc as bacc
import concourse.mybir as mybir
from concourse.bass_utils import run_bass_kernel_spmd
from concourse.tile import TileContext

B, N, D = 4, 4096, 1024
N_CORES = 8
LANES = B * D                        # 4096 independent (b, d) lanes
LANES_PER_CORE = LANES // N_CORES    # 512
P = 128                              # SBUF partitions
LPP = LANES_PER_CORE // P            # 4 lanes per partition
NC = LPP * N                         # 16384 columns per partition

TRACE = False       # test harness sets True to capture a neuron-profile trace
USE_BF16 = True     # bf16 inputs: quantization ~2e-3 rel, halves load bytes
BF16_OUT = True     # bf16 output stores: halves store bytes
_result_info = {}   # exec_time_ns / trace path from the last run

# Column segment sizes (sum = NC). Small head segments let the scan start
# as soon as the first columns land; the short tail segment keeps the
# final store off the critical path.
import os as _os

_SEGS = [int(s) for s in _os.environ.get(
    "SEGS", "2048,2048,4096,4096,3584,512"
).split(",")]
assert sum(_SEGS) == NC
_STORE_ENG = _os.environ.get("STORE_ENG", "scalar")
# Segments >= this index load g and x packed in a single transfer
# (measured no faster than two 8KB-line transfers; disabled by default).
_PACK_FROM = int(_os.environ.get("PACK_FROM", "99"))
# Max columns per scan instruction / store transfer (loads stay whole-seg).
_SUBSEG = int(_os.environ.get("SUBSEG", "2048"))


def _build() -> bacc.Bacc:
    in_dt = mybir.dt.bfloat16 if USE_BF16 else mybir.dt.float32
    out_dt = mybir.dt.bfloat16 if BF16_OUT else mybir.dt.float32
    nc = bacc.Bacc()
    # One contiguous DRAM tensor per segment: every DMA source/dest is a
    # single dense block, which keeps the queues at full descriptor
    # efficiency (column-slicing one big [P, NC] tensor dropped the load
    # rings to ~140 B/ns; dense blocks run ~290 B/ns). Segments >= _PACK_FROM
    # pack g and x into one [P, 2*seg] tensor (row = g||x): double the
    # per-partition line length and half the transfer count.
    gs, xs, gxs = [], [], []
    for k, seg in enumerate(_SEGS):
        if k >= _PACK_FROM:
            gxs.append(
                nc.dram_tensor(f"gx{k}", [P, 2 * seg], in_dt, kind="ExternalInput")
            )
            gs.append(None)
            xs.append(None)
        else:
            gxs.append(None)
            gs.append(
                nc.dram_tensor(f"g{k}", [P, seg], in_dt, kind="ExternalInput")
            )
            xs.append(
                nc.dram_tensor(f"x{k}", [P, seg], in_dt, kind="ExternalInput")
            )
    os_ = [
        nc.dram_tensor(f"o{k}", [P, seg], out_dt, kind="ExternalOutput")
        for k, seg in enumerate(_SEGS)
    ]
    M = mybir.AluOpType.mult
    A = mybir.AluOpType.add
    with TileContext(nc) as tc:
        with tc.tile_pool(name="pool", bufs=1) as pool:
            gts, xts, gxts, ots = [], [], [], []
            for k, seg in enumerate(_SEGS):
                if k >= _PACK_FROM:
                    gxt = pool.tile([P, 2 * seg], in_dt, name=f"gxt{k}")
                    gxts.append(gxt)
                    gts.append(gxt[:, 0:seg])
                    xts.append(gxt[:, seg : 2 * seg])
                else:
                    gxts.append(None)
                    gts.append(pool.tile([P, seg], in_dt, name=f"gt{k}"))
                    xts.append(pool.tile([P, seg], in_dt, name=f"xt{k}"))
                ots.append(pool.tile([P, seg], out_dt, name=f"ot{k}"))
            # Issue every load up front: g on the sync HWDGE ring, x on the
            # scalar HWDGE ring. All tiles coexist in SBUF (96 KiB/partition),
            # so nothing waits on a buffer release.
            x_head = int(_os.environ.get("X_HEAD", "2"))
            for k, seg in enumerate(_SEGS):
                if k >= _PACK_FROM:
                    nc.sync.dma_start(out=gxts[k][:, :], in_=gxs[k][:, :])
                elif k < x_head:
                    # Head: g and x on different rings so both queues ramp
                    # concurrently and the first segment lands sooner.
                    nc.sync.dma_start(out=gts[k][:, :], in_=gs[k][:, :])
                    nc.scalar.dma_start(out=xts[k][:, :], in_=xs[k][:, :])
                else:
                    nc.sync.dma_start(out=gts[k][:, :], in_=gs[k][:, :])
                    nc.sync.dma_start(out=xts[k][:, :], in_=xs[k][:, :])
            # Chained scans; carry crosses segment boundaries through the
            # previous segment's last output column (bf16 rounding there is
            # far inside the error budget). Lane resets happen wherever the
            # host zeroed the gate. Stores ride the gpsimd SWDGE ring so
            # they never contend with load dispatch.
            engs = {
                "gpsimd": nc.gpsimd,
                "scalar": nc.scalar,
                "sync": nc.sync,
            }
            store_eng = engs[_STORE_ENG]
            # Scan/store in sub-pieces of at most _SUBSEG columns while
            # keeping the big load transfers: stores then stream during the
            # scan instead of piling up behind the last one, shortening the
            # post-scan drain. The carry chains through the previous piece's
            # last output column.
            prev = None
            for k, seg in enumerate(_SEGS):
                c = 0
                while c < seg:
                    sub = min(_SUBSEG, seg - c)
                    init = 0.0 if prev is None else prev
                    nc.vector.tensor_tensor_scan(
                        ots[k][:, c : c + sub],
                        gts[k][:, c : c + sub],
                        xts[k][:, c : c + sub],
                        init,
                        M,
                        A,
                    )
                    store_eng.dma_start(
                        out=os_[k][:, c : c + sub], in_=ots[k][:, c : c + sub]
                    )
                    prev = ots[k][:, c + sub - 1 : c + sub]
                    c += sub
    nc.compile()
    return nc


def kernel(gates: np.ndarray, inputs: np.ndarray) -> np.ndarray:
    gates = np.asarray(gates, dtype=np.float32)
    inputs = np.asarray(inputs, dtype=np.float32)

    # Host-side shard: (B, N, D) -> lane-major (B*D, N); row b*D + d is the
    # contiguous time series of lane (b, d). The first gate of every lane
    # multiplies the zero initial state, so it is dead — zero it to make
    # the scan state reset at lane boundaries after concatenation.
    gt = np.ascontiguousarray(gates.transpose(0, 2, 1)).reshape(LANES, N)
    xt = np.ascontiguousarray(inputs.transpose(0, 2, 1)).reshape(LANES, N)
    gt[:, 0] = 0.0
    if USE_BF16:
        import ml_dtypes

        gt = gt.astype(ml_dtypes.bfloat16)
        xt = xt.astype(ml_dtypes.bfloat16)

    # Per core: [512, N] -> [LPP, P, N] -> [P, LPP, N] -> [P, NC]: partition
    # p holds lanes {base + p, base + P + p, ...} concatenated in time.
    # Each column segment ships as its own contiguous array.
    bounds = np.cumsum([0] + _SEGS)
    in_maps = []
    for c in range(N_CORES):
        rows = slice(c * LANES_PER_CORE, (c + 1) * LANES_PER_CORE)
        gc = gt[rows].reshape(LPP, P, N).transpose(1, 0, 2).reshape(P, NC)
        xc = xt[rows].reshape(LPP, P, N).transpose(1, 0, 2).reshape(P, NC)
        m = {}
        for k in range(len(_SEGS)):
            sl = slice(bounds[k], bounds[k + 1])
            if k >= _PACK_FROM:
                m[f"gx{k}"] = np.ascontiguousarray(
                    np.concatenate([gc[:, sl], xc[:, sl]], axis=1)
                )
            else:
                m[f"g{k}"] = np.ascontiguousarray(gc[:, sl])
                m[f"x{k}"] = np.ascontiguousarray(xc[:, sl])
        in_maps.append(m)

    nc = _build()
    res = run_bass_kernel_spmd(
        nc, in_maps, core_ids=list(range(N_CORES)), trace=TRACE
    )
    _result_info["exec_time_ns"] = res.exec_time_ns
    _result_info["mean_exec_time_ns"] = res.mean_exec_time_ns
    _result_info["profile_json"] = res.profile_json
    _result_info["trace"] = (
        res.instructions_and_trace[1] if res.instructions_and_trace else None
    )

    # Undo the per-core packing: concat segments -> [P, NC] -> [P, LPP, N]
    # -> [LPP, P, N] -> [512, N], then stack cores back to (LANES, N).
    parts = []
    for c in range(N_CORES):
        oc = np.concatenate(
            [
                res.results[c][f"o{k}"].astype(np.float32, copy=False)
                for k in range(len(_SEGS))
            ],
            axis=1,
        )
        parts.append(
            oc.reshape(P, LPP, N).transpose(1, 0, 2).reshape(LANES_PER_CORE, N)
        )
    out_t = np.concatenate(parts, axis=0)  # (LANES, N)
    return np.ascontiguousarray(out_t.reshape(B, D, N).transpose(0, 2, 1))
